# revision 5
# baseline (speedup 1.0000x reference)
"""ConvTranspose2d(64->64, k=3, s=1, p=0) on (2, 64, 1024, 1024) fp32.

out[b, o, p, q] = sum_{c,kh,kw} weight[c, o, kh, kw] * x[b, c, p-kh, q-kw]
out shape (2, 64, 1026, 1026).

Strategy (8 NeuronCores, data parallel over batch x H, bf16 on device):
  - Each core handles one batch and a quarter of the 513 output-row pairs.
  - Output rows are processed in PAIRS (2j, 2j+1). Input-row pairs are packed
    host-side as bf16 "U tiles" (partition = 64*u + c; u = row-of-pair,
    c = channel) laid out partition-major in DRAM: xs[p, t*WP + col], with 2
    zero pad columns each side (WP = 1028) and a zero halo tile at t=0.
  - Blocks of T=16 pairs: one input DMA per block loads T+1 U tiles
    (contiguous per partition) on the SP HWDGE queue; one output DMA per
    block stores T row-pairs on the Activation HWDGE queue (split queues).
  - Per pair and output chunk (3 x 342 cols), six K=128 bf16 matmuls
    accumulate in PSUM: stationary blocks A_s = [[W0s, W1s], [0, W0s]] vs
    the pair's own U tile and B_s = [[W2s, 0], [W1s, W2s]] vs the previous
    U tile (s = kw shift via the baked-in pad columns). PSUM fp32 accumulate;
    Activation-engine copies cast PSUM fp32 -> bf16 into the block
    output tile (keeps DVE free; DVE copies were serializing with PE).
  - Host packs/unpacks fp32 <-> bf16; rel err ~3e-3 (tolerance 2e-2).
"""

import numpy as np
import ml_dtypes

BF16NP = ml_dtypes.bfloat16

B = 2
C = 64
H = 1024
W = 1024
HO = 1026
WO = 1026
WP = W + 4  # 2 zero pad cols each side
NPAIR = 129  # output row pairs computed per core
NTILE = NPAIR + 1  # U tiles per core incl. leading halo tile
J0S = (0, 128, 256, 384)  # first output pair per core (within a batch)
VALID = (128, 128, 128, 129)  # pairs consumed from each core
CHUNKS = ((0, 342), (342, 342), (684, 342))
T = 16  # max pairs per DMA block
# Tapered block schedule: small blocks at the start (compute begins after a
# short first DMA) and at the end (short final output-DMA tail).
SIZES = (2, 4, 8) + (16,) * 6 + (12, 6, 1)

U_BUFS = 2
O_BUFS = 2
PS_BUFS = 3

_CACHE = {}


def _build(npair=NPAIR, reps=1, t_blk=T, u_bufs=U_BUFS, o_bufs=O_BUFS,
           ps_bufs=PS_BUFS, copy_eng="scalar", out_eng="scalar"):
    import concourse.bacc as bacc
    import concourse.mybir as mybir
    from concourse.tile import TileContext

    F32 = mybir.dt.float32
    BF16 = mybir.dt.bfloat16
    ntile = npair + 1

    blocks = []
    j0 = 0
    if npair == NPAIR:
        for nb in SIZES:
            blocks.append((j0, nb))
            j0 += nb
    else:
        while j0 < npair:
            nb = min(t_blk, npair - j0)
            blocks.append((j0, nb))
            j0 += nb

    nc = bacc.Bacc()
    xs = nc.dram_tensor("xs", [128, ntile * WP], BF16, kind="ExternalInput")
    ws = nc.dram_tensor("ws", [128, 768], BF16, kind="ExternalInput")
    outs = nc.dram_tensor("outs", [128, npair * WO], BF16, kind="ExternalOutput")
    with TileContext(nc) as tc:
        with (
            tc.tile_pool(name="w", bufs=1) as wpool,
            tc.tile_pool(name="u", bufs=u_bufs) as up,
            tc.tile_pool(name="ob", bufs=o_bufs) as ob,
            tc.tile_pool(name="ps", bufs=ps_bufs, space="PSUM") as pp,
        ):
            wsb = wpool.tile([128, 768], BF16)
            nc.scalar.dma_start(out=wsb, in_=ws[:, :])
            oeng = getattr(nc, out_eng)
            ceng = getattr(nc, copy_eng)
            for _ in range(reps):
                for j0, nb in blocks:
                    ub = up.tile([128, (t_blk + 1) * WP], BF16, tag="u")
                    nc.sync.dma_start(
                        out=ub[:, : (nb + 1) * WP],
                        in_=xs[:, j0 * WP : (j0 + nb + 1) * WP],
                    )
                    osb = ob.tile([128, t_blk * WO], BF16, tag="ob")
                    for jj in range(nb):
                        for ci, (n0, nch) in enumerate(CHUNKS):
                            ps = pp.tile(
                                [128, nch],
                                F32,
                                tag=f"c{ci}",
                                name=f"ps{ci}",
                                bufs=ps_bufs if ci < 2 else min(ps_bufs, 2),
                            )
                            k = 0
                            for g in (0, 1):
                                off = (jj + 1 - g) * WP
                                for s in range(3):
                                    i0 = (g * 3 + s) * 128
                                    a0 = off + n0 + 2 - s
                                    nc.tensor.matmul(
                                        ps[:, :],
                                        wsb[:, i0 : i0 + 128],
                                        ub[:, a0 : a0 + nch],
                                        start=(k == 0),
                                        stop=(k == 5),
                                    )
                                    k += 1
                            dst = osb[:, jj * WO + n0 : jj * WO + n0 + nch]
                            if ci < 2:
                                nc.scalar.copy(out=dst, in_=ps[:, :])
                            else:
                                nc.vector.tensor_copy(out=dst, in_=ps[:, :])
                    oeng.dma_start(
                        out=outs[:, j0 * WO : (j0 + nb) * WO],
                        in_=osb[:, : nb * WO],
                    )
    nc.compile()
    return nc


def _pack_weight(weight):
    """weight (64, 64, 3, 3) fp32 -> (128, 768) bf16 stationary blocks.

    ws[64*u + c, (3*g + s)*128 + 64*v + o] = weight[c, o, v - u + 2*g, s]
    when 0 <= v - u + 2*g <= 2 else 0.
    """
    wsb = np.zeros((128, 768), np.float32)
    for g in (0, 1):
        for s in range(3):
            col0 = (3 * g + s) * 128
            for u in (0, 1):
                for v in (0, 1):
                    kh = v - u + 2 * g
                    if 0 <= kh <= 2:
                        wsb[64 * u : 64 * u + 64, col0 + 64 * v : col0 + 64 * v + 64] = (
                            weight[:, :, kh, s]
                        )
    return wsb.astype(BF16NP)


def _pack_core_input(xb, j0):
    """xb (64, 1024, 1024) fp32 -> xs (128, NTILE*WP) bf16 partition-major.

    Tile t holds U_{j0+t-1}: rows 2*(j0+t-1) and +1 (zero outside [0, H)),
    2 zero pad columns both sides. Partition 64*u + c; per-partition layout
    is NTILE consecutive row-segments of width WP.
    """
    arr = np.zeros((128, NTILE, WP), BF16NP)
    r0 = 2 * j0 - 2  # source row of tile 0, u=0
    lo = max(0, r0)
    hi = min(H, r0 + 2 * NTILE)
    l0 = lo - r0  # always even
    ev = xb[:, lo:hi:2, :]
    od = xb[:, lo + 1 : hi : 2, :]
    arr[0:64, l0 // 2 : l0 // 2 + ev.shape[1], 2 : 2 + W] = ev
    arr[64:128, l0 // 2 : l0 // 2 + od.shape[1], 2 : 2 + W] = od
    return arr.reshape(128, NTILE * WP)


def _unpack_output(results):
    out = np.empty((B, C, HO, WO), np.float32)
    for core in range(8):
        b, k = divmod(core, 4)
        nv = VALID[k]
        # [128, NPAIR*WO] bf16 -> (v, o, j, col) -> (o, 2j+v, col)
        a = (
            results[core]["outs"]
            .astype(np.float32)
            .reshape(2, 64, NPAIR, WO)
            .transpose(1, 2, 0, 3)
            .reshape(64, NPAIR * 2, WO)
        )
        out[b, :, 2 * J0S[k] : 2 * (J0S[k] + nv), :] = a[:, : 2 * nv]
    return out


def kernel(x, weight):
    from concourse.bass_utils import run_bass_kernel_spmd

    x = np.ascontiguousarray(x, dtype=np.float32)
    weight = np.ascontiguousarray(weight, dtype=np.float32)

    if "nc" not in _CACHE:
        _CACHE["nc"] = _build()
    nc = _CACHE["nc"]

    wsb = _pack_weight(weight)
    in_maps = []
    for core in range(8):
        b, k = divmod(core, 4)
        in_maps.append({"xs": _pack_core_input(x[b], J0S[k]), "ws": wsb})

    res = run_bass_kernel_spmd(nc, in_maps, core_ids=list(range(8)))

    return _unpack_output(res.results)
